# revision 54
# baseline (speedup 1.0000x reference)
"""Causal multi-head attention (B=16, T=1024, E=1024, H=16, Dh=64) on 8 TRN2
NeuronCores.

Sharding: data-parallel over batch -- 2 batch elements per core, weights
replicated, no collectives. Host pre-transposes x and packs weights; each core
runs an identical Bass/Tile program on its shard.

Per-core dataflow (all in "transposed" orientation so no on-chip transposes
are ever needed):
  x^T [E,T] (host)   --matmul-->  Q^T,K^T [Dh,T] per head (head-pairs packed
                                  into 128 partitions; 1/sqrt(Dh) folded into
                                  the Q PSUM->SBUF copy)
                     --matmul-->  V [T,Dh] per head (+ ones column)
  S^T[tk,tq] = (K^T tile).T @ Q^T  per key-tile, causal tiles skipped
  P^T = exp(S^T) on ScalarE (scores are O(1): no max subtraction needed);
        diagonal tiles masked by 0/1 multiply
  O'^T[65,tq] += (V'|1).T @ P^T   -- row 64 accumulates the softmax denom
  Y^T = O'^T[0:64] * bcast(1/denom)
  out[t,E] = Y^T.T @ Wo + bo

Scheduling notes (hard-won on HW):
  - engine APs need partition base in {0,32,64,96}; partition_broadcast reads
    physical partition 0 regardless of the AP; gpsimd + custom-DVE ops
    (reciprocal_approx_fast) silently misread NONZERO partition bases on HW
    (CoreSim hides it), and approx_fast must also be out-of-place.
  - HAM unthrottles the PE (1.2 -> 2.4 GHz) only on a fully-busy 3.4us
    window, so each pair's Q/K projection matmuls, the previous batch's
    out-proj blocks (3/pair over pairs 0-5) and the next batch's x-load +
    V-projection (pairs 6-7) are drip-fed into the attention stream as
    dense PE filler.
  - the attention sub-block loop is software-pipelined a half step (scores
    for sub n+1 issue before PV of sub n) to cover the exp latency; deeper
    lookahead deadlocks the Tile scheduler's finite wait queues.
  - all PSUM->SBUF copies run on the DVE (tensor_copy), keeping the scalar
    engine dedicated to the exp stream, which paces the attention phase.
  - normalization (approx-reciprocal + stage + broadcast + 256-col muls) is
    drip-emitted between sub-steps; the final batch pre-starts 8 out-proj
    j=0..5 accumulations (4 psc + 4 halves of freed PV PSUM banks) as PE
    cover for the last normalize chains.
"""
import numpy as np
import ml_dtypes

import concourse.bass as bass
import concourse.mybir as mybir
import concourse.tile as tile
from concourse import bacc
from concourse.bass_utils import run_bass_kernel_spmd

B, T, E = 16, 1024, 1024
H, Dh = 16, 64
NCORES = 8
BL = B // NCORES          # batches per core
P = 128                   # partitions
ET = E // P               # 8 tiles along E / token / hd dims
HP = H // 2               # 8 head-pairs
BF = mybir.dt.bfloat16
F8 = mybir.dt.float8e4
F32 = mybir.dt.float32
AF = mybir.ActivationFunctionType
DR = mybir.MatmulPerfMode.DoubleRow
# Wq/Wk are uploaded as fp8 scaled by 32 (half their values sit below
# e4m3's normal range otherwise); 0.125 = 1/sqrt(Dh) is NOT folded into
# Wq anymore.  S' = (32 q)·(32 k) = 1024·qk, so exp gets scale
# 0.125/1024 for free via the activation scale operand.
WSCALE = 32.0
ESCALE = 0.125 / (WSCALE * WSCALE)

_CACHE = {}


def _pieces(i):
    """Column pieces of [128*i, 1024) that do not cross the 512 PSUM-bank
    boundary."""
    if i < 4:
        return [(128 * i, 512), (512, 1024)]
    return [(128 * i, 1024)]


def _build(dbg=False):
    nc = bacc.Bacc("TRN2", target_bir_lowering=False, debug=False,
                   num_devices=NCORES)

    dbg_out = {}
    if dbg:
        for name, shape, dt in [
            ("d_qT", [P, HP, T], BF), ("d_kT", [P, HP, T], BF),
            ("d_v", [P, ET, H, Dh + 1], BF), ("d_pt", [ET, P, T], BF),
            ("d_op", [P, T], F32),
            ("d_r1", [1, T], F32), ("d_rb", [Dh, T], F32),
            ("d_yT", [P, HP, T], BF),
        ]:
            dbg_out[name] = nc.dram_tensor(name, shape, dt,
                                           kind="ExternalOutput").ap()

    xT = nc.dram_tensor("xT", [BL, E, T], BF, kind="ExternalInput").ap()
    xT8 = nc.dram_tensor("xT8", [BL, E, T], F8, kind="ExternalInput").ap()
    wq = nc.dram_tensor("wq", [E, H * Dh], F8, kind="ExternalInput").ap()
    wk = nc.dram_tensor("wk", [E, H * Dh], F8, kind="ExternalInput").ap()
    wv = nc.dram_tensor("wv", [E, H * Dh], BF, kind="ExternalInput").ap()
    wo = nc.dram_tensor("wo", [H * Dh, E], BF, kind="ExternalInput").ap()
    borep = nc.dram_tensor("borep", [P, E], BF, kind="ExternalInput").ap()
    mask01 = nc.dram_tensor("mask01", [P, P], BF, kind="ExternalInput").ap()
    out = nc.dram_tensor("out", [BL, T, E], F32, kind="ExternalOutput").ap()

    with tile.TileContext(nc) as tc:
        with (
            tc.tile_pool(name="consts", bufs=1) as cpool,
            tc.tile_pool(name="xp", bufs=2) as xpool,
            tc.tile_pool(name="xp8", bufs=2) as x8pool,
            tc.tile_pool(name="qk", bufs=2) as qkpool,
            tc.tile_pool(name="vp2", bufs=2) as vpool,
            tc.tile_pool(name="vy", bufs=2) as vypool,
            tc.tile_pool(name="pt", bufs=6) as ptpool,
            tc.tile_pool(name="sm", bufs=2) as spool,
            tc.tile_pool(name="dn", bufs=1) as dnpool,
            tc.tile_pool(name="ob", bufs=3) as opool,
            tc.tile_pool(name="pso", bufs=2, space="PSUM") as pso,
            tc.tile_pool(name="psc", bufs=4, space="PSUM") as psc,
        ):
            # DMA order matters for the startup critical path: V-projection
            # only needs Wv + xT, so those go first; Wo/bias are not needed
            # until the output projection
            wq_sb = cpool.tile([P, ET, H * Dh], F8, tag="wq")
            wk_sb = cpool.tile([P, ET, H * Dh], F8, tag="wk")
            wv_sb = cpool.tile([P, ET, H * Dh], BF, tag="wv")
            wo_sb = cpool.tile([P, ET, E], BF, tag="wo")
            # startup order: V-path inputs (wv in halves, then xT in
            # t-chunks) go first -- V-projection blocks are the PE's
            # startup filler and the first one only needs wv's first half
            # plus xT chunk 0; Q/K-proj inputs stream in underneath
            # weights ride the scalar engine's DMA ring so they stream
            # concurrently with the xT chunks on the sync ring -- the first
            # V matmul needs wv half 0 + xT chunk 0, both ~3us in
            wvr = wv.rearrange("(n p) c -> p n c", p=P)
            nc.scalar.dma_start(wv_sb[:, :, 0:512], wvr[:, :, 0:512])

            xT_tiles = {}
            xT8_tiles = {}
            v_tiles = {}

            def load_blocks(b):
                """xT load + V-projection for batch b as dense PE filler
                blocks (also usable as pending entries during the previous
                batch's last attention pairs)."""
                def ld(b=b):
                    # xT arrives in t-chunks so V block t only waits for
                    # its own chunk (first V matmul ~8us instead of ~24us)
                    xT_tiles[b] = xpool.tile([P, ET, T], BF, tag="xT",
                                             name=f"xT{b}")
                    xsrc = xT[b].rearrange("(n p) c -> p n c", p=P)
                    for t in range(ET):
                        ts_ = slice(128 * t, 128 * (t + 1))
                        nc.sync.dma_start(xT_tiles[b][:, :, ts_],
                                          xsrc[:, :, ts_])
                    xT8_tiles[b] = x8pool.tile([P, ET, T], F8, tag="xT8",
                                               name=f"xT8_{b}")
                    nc.sync.dma_start(
                        xT8_tiles[b][:],
                        xT8[b].rearrange("(n p) c -> p n c", p=P))
                    v_tiles[b] = vpool.tile([P, ET, H, Dh + 1], BF, tag="v",
                                            name=f"v{b}")
                    nc.vector.memset(v_tiles[b][:, :, :, Dh], 1.0)
                blocks = [ld]
                for n2 in range(2):
                    for t in range(ET):
                        def vblk(t=t, n2=n2, b=b):
                            cs = slice(512 * n2, 512 * (n2 + 1))
                            vp = psc.tile([P, 512], F32, tag="pc",
                                          name=f"vp{b}_{t}_{n2}")
                            for i in range(ET):
                                nc.tensor.matmul(
                                    vp[:],
                                    lhsT=xT_tiles[b][:, i,
                                                     128 * t:128 * (t + 1)],
                                    rhs=wv_sb[:, i, cs],
                                    start=(i == 0), stop=(i == ET - 1),
                                )
                            nc.vector.tensor_copy(
                                v_tiles[b][:, t, 8 * n2:8 * (n2 + 1), 0:Dh],
                                vp[:].rearrange("p (h d) -> p h d", d=Dh),
                            )
                        blocks.append(vblk)
                return blocks

            blocks0 = load_blocks(0)
            blocks0[0]()  # xT chunks + x8 on the sync ring
            nc.scalar.dma_start(wv_sb[:, :, 512:1024], wvr[:, :, 512:1024])
            nc.scalar.dma_start(wq_sb[:],
                                wq.rearrange("(n p) c -> p n c", p=P))
            nc.scalar.dma_start(wk_sb[:],
                                wk.rearrange("(n p) c -> p n c", p=P))
            mask_sb = cpool.tile([P, P], BF, tag="mask")
            nc.scalar.dma_start(mask_sb[:], mask01)
            for blk in blocks0[1:]:
                blk()
            nc.sync.dma_start(wo_sb[:], wo.rearrange("(n p) c -> p n c", p=P))
            borep_sb = cpool.tile([P, E], BF, tag="bo")
            nc.sync.dma_start(borep_sb[:], borep)

            pending = []

            def drain(n):
                for _ in range(min(n, len(pending))):
                    pending.pop(0)()

            qk_tiles = {}
            deferred_op = []

            for b in range(BL):
                xT_sb = xT_tiles[b]
                v_sb = v_tiles[b]

                # ---- Q^T / K^T projections, emitted as closures so pair
                # pp's projection interleaves into pair pp-1's attention.
                # Per-pair [P,T] tiles (bufs=2): finer Tile deps + 24KB SBUF
                # saved vs whole-batch [P,HP,T] tiles ----
                def proj_subblocks(pp, b=b):
                    # fp8 DoubleRow: each pass consumes TWO 128-deep E tiles
                    # (lhsT [128,2,128], rhs [128,2,512]), halving PE time
                    # vs bf16.  Q^T/K^T land in bf16 [P,T] per head-pair;
                    # the scores matmul stays bf16 (a single 64-deep pass --
                    # DoubleRow only pays when it halves accumulation
                    # passes, so fp8 there buys nothing).
                    blocks = []
                    for (lbl, w_sb) in (("q", wq_sb), ("k", wk_sb)):
                        for n2 in range(2):
                            def blk(lbl=lbl, w_sb=w_sb, n2=n2, pp=pp, b=b):
                                if (b, pp) not in qk_tiles:
                                    qk_tiles[(b, pp)] = (
                                        qkpool.tile([P, T], BF, tag="q",
                                                    name=f"q{b}_{pp}"),
                                        qkpool.tile([P, T], BF, tag="k",
                                                    name=f"k{b}_{pp}"))
                                dst = qk_tiles[(b, pp)][0 if lbl == "q"
                                                        else 1]
                                cs = slice(512 * n2, 512 * (n2 + 1))
                                x8_sb = xT8_tiles[b]
                                pj = psc.tile(
                                    [P, 512], F32, tag="pc",
                                    name=f"pj{b}_{pp}_{n2}_{lbl}")
                                for i in range(0, ET, 2):
                                    nc.tensor.matmul(
                                        pj[:],
                                        lhsT=w_sb[:, i:i + 2,
                                                  128 * pp:128 * (pp + 1)],
                                        rhs=x8_sb[:, i:i + 2, cs],
                                        start=(i == 0), stop=(i == ET - 2),
                                        perf_mode=DR,
                                    )
                                nc.vector.tensor_copy(dst[:, cs], pj[:])
                            blocks.append(blk)
                    return blocks

                for blk in proj_subblocks(0):
                    blk()

                # ---- attention: pairs of heads, drip-scheduled extras ----
                # per-pair yT tiles, double-buffered across batches: out-proj
                # j only waits on pair j's normalize, and the previous
                # batch's deferred out-proj never blocks this batch's writes
                yTp = [vypool.tile([P, T], BF, tag=f"y{hp}",
                                   name=f"y{b}_{hp}") for hp in range(HP)]

                def yT_ap(hp):
                    return yTp[hp][:, :]
                # den: PV-accumulated softmax denominators (2 rotating quad
                # slots); den_r: their reciprocals. approx_fast is ~5x faster
                # than reciprocal() but MUST be out-of-place: in-place it
                # returns garbage on HW (verified; CoreSim hides it)
                den = dnpool.tile([P, 2, T], F32, tag="den")
                den_r = dnpool.tile([P, 2, T], F32, tag="denr")
                nc.vector.memset(den[:], 1.0)

                def queue_normalize(g, b=b, yT_ap=yT_ap, den=den,
                                    half=None):
                    # half=0/1 reciprocates only partitions [0:64)/[64:128)
                    # of the slot (heads 4g..4g+1 / 4g+2..4g+3) so the last
                    # quad can normalize pair-by-pair instead of in one
                    # end-of-batch burst
                    # approx_fast also breaks on HW with a nonzero partition
                    # offset, so always recip the full 128 partitions; for
                    # the half= calls the other half recomputes identical
                    # values (benign)
                    for c in range(4):
                        def recip_chunk(g=g, c=c):
                            nc.vector.reciprocal_approx_fast(
                                den_r[0:P, g % 2, 256 * c:256 * (c + 1)],
                                den[0:P, g % 2, 256 * c:256 * (c + 1)])
                        pending.append(recip_chunk)
                    if dbg and b == 0 and g == 0 and half is None:
                        pending.append(lambda: nc.sync.dma_start(
                            dbg_out["d_r1"], den[0:1, 0, :]))
                    heads = range(4 * g, 4 * g + 4) if half is None else \
                        range(4 * g + 2 * half, 4 * g + 2 * half + 2)
                    for h in heads:
                        holder = {}

                        def stage(h=h, b=b, holder=None):
                            hp, po = h // 2, Dh * (h % 2)
                            pb = 32 * (h % 4)
                            r1 = spool.tile([1, T], BF, tag="r1",
                                            name=f"r1_{b}_{h}")
                            nc.scalar.activation(
                                r1[:], den_r[pb:pb + 1, (h // 4) % 2, :],
                                AF.Copy)
                            rb = spool.tile([P, T], BF, tag="rb",
                                            name=f"rb_{b}_{h}")
                            nc.gpsimd.partition_broadcast(rb[:], r1[:])
                            holder['rb'] = rb

                        pending.append(
                            lambda h=h, holder=holder: stage(h, b, holder))
                        # normalize muls chunked to 256 cols so no single
                        # DVE op delays the mask->PV chain by more than
                        # ~0.5us
                        for c in range(4):
                            def mul_chunk(h=h, holder=holder, c=c):
                                hp, po = h // 2, Dh * (h % 2)
                                ap = yT_ap(hp)
                                sl = slice(256 * c, 256 * (c + 1))
                                nc.vector.tensor_mul(
                                    ap[po:po + Dh, sl],
                                    ap[po:po + Dh, sl],
                                    holder['rb'][po:po + Dh, sl])
                            pending.append(mul_chunk)

                for hp in range(HP):
                    if deferred_op and hp >= 1:
                        # previous batch's out-proj blocks as dense PE
                        # filler -- NOT at pair 0: every block reads all 8
                        # yT pairs, and the previous batch's last normalize
                        # only clears during pair 0, so eager drips stall
                        # the PE there
                        take = 3 if hp <= 2 else 2
                        pending.extend(deferred_op[:take])
                        del deferred_op[:take]
                    if hp + 1 < HP:
                        # front of the queue: pair hp+1's projection must
                        # finish within this pair's attention
                        pending[0:0] = proj_subblocks(hp + 1)
                    if b + 1 < BL:
                        # next batch's x load + V projection: V blocks have
                        # no upstream compute deps, so they carry pair 0
                        # (where out-proj can't run yet) and then 2/pair
                        if hp == 0:
                            next_blocks = load_blocks(b + 1)
                            pending.extend(next_blocks[:5])
                        else:
                            pending.extend(next_blocks[2 * hp + 3:
                                                       2 * hp + 5])
                    ops = [pso.tile([P, 1024], F32, tag="op",
                                    name=f"op{b}_{hp}_{s}") for s in range(2)]

                    def emit_scores_sub(i, sub, hp=hp, b=b):
                        # per-piece P tiles: the PV matmul for the non-
                        # diagonal piece only waits on its own exp, not on
                        # the diagonal piece's exp+mask
                        qTp, kTp = qk_tiles[(b, hp)]
                        po = Dh * sub
                        pts = []
                        for (a0, a1) in _pieces(i):
                            w = a1 - a0
                            ptp = ptpool.tile([P, 512], BF, tag="pt",
                                              name=f"pt{b}_{hp}_{i}_{sub}_{a0}")
                            sp_ = psc.tile([P, 512], F32, tag="pc",
                                           name=f"sp{b}_{hp}_{i}_{sub}_{a0}")
                            nc.tensor.matmul(
                                sp_[:, 0:w],
                                lhsT=kTp[po:po + Dh,
                                         128 * i:128 * (i + 1)],
                                rhs=qTp[po:po + Dh, a0:a1],
                                start=True, stop=True,
                            )
                            nc.scalar.activation(ptp[:, 0:w],
                                                 sp_[:, 0:w], AF.Exp,
                                                 scale=ESCALE)
                            if a0 == 128 * i:
                                # diagonal 128-col block lives at the start
                                # of the first piece
                                nc.vector.tensor_mul(ptp[:, 0:128],
                                                     ptp[:, 0:128],
                                                     mask_sb[:])
                            pts.append((a0, a1, ptp))
                        return pts

                    def emit_pv_sub(i, sub, pts, hp=hp, ops=ops, v_sb=v_sb,
                                    b=b):
                        h = 2 * hp + sub
                        for (a0, a1, ptp) in pts:
                            nc.tensor.matmul(
                                ops[sub][0:Dh + 1, a0:a1],
                                lhsT=v_sb[:, i, h, :],
                                rhs=ptp[:, 0:a1 - a0],
                                start=(i == 0), stop=(i == ET - 1),
                                skip_group_check=True,
                            )
                        if i == ET - 1:
                            # yT+den copies dripped into the next pair's
                            # drain stream: emitted inline they burst ~4us
                            # of DVE right when the next pair's mask muls
                            # need it (the pso pool is double-buffered, so
                            # the psum only has to free a pair later)
                            def pv_out_copies(hp=hp, sub=sub, h=h,
                                              ops=ops):
                                po = Dh * sub
                                nc.vector.tensor_copy(
                                    yT_ap(hp)[po:po + Dh, :],
                                    ops[sub][0:Dh, :])
                                pb = 32 * (h % 4)
                                nc.vector.tensor_copy(
                                    den[pb:pb + 1, (h // 4) % 2, :],
                                    ops[sub][Dh:Dh + 1, :])
                            pending.append(pv_out_copies)
                            if dbg and b == 0 and h == 0:
                                opc = dnpool.tile([P, T], F32,
                                                  tag="dbg_op")
                                nc.vector.tensor_copy(opc[:],
                                                      ops[sub][:])
                                nc.sync.dma_start(dbg_out["d_op"],
                                                  opc[:])

                    # half-step software pipeline: scores for sub-block n+1
                    # issue on the PE before the PV matmuls of sub-block n,
                    # covering the exp+mask latency without over-filling the
                    # engines' wait queues (full-step lookahead deadlocks the
                    # Tile scheduler)
                    dr = 2 if (b == BL - 1 and hp >= HP - 2) else 1
                    prev = None
                    for i in range(ET):
                        for sub in (0, 1):
                            pt = emit_scores_sub(i, sub)
                            drain(dr)
                            if prev is not None:
                                emit_pv_sub(*prev)
                            prev = (i, sub, pt)
                    drain(dr)
                    emit_pv_sub(*prev)
                    if hp == HP - 2:
                        queue_normalize(3, half=0)
                    elif hp == HP - 1:
                        queue_normalize(3, half=1)
                    elif hp % 2 == 1:
                        queue_normalize(hp // 2)

                # emission order IS dependency order under Tile's tracer:
                # all normalize muls must be emitted before out-proj reads yT
                drain(len(pending))

                # ---- output projection + bias ----
                def oproj_mm(t, n2, acc, j0, j1, b=b, yTp=yTp):
                    cs = slice(512 * n2, 512 * (n2 + 1))
                    for j in range(j0, j1):
                        nc.tensor.matmul(
                            acc,
                            lhsT=yTp[j][:, 128 * t:128 * (t + 1)],
                            rhs=wo_sb[:, j, cs],
                            start=(j == 0), stop=(j == ET - 1),
                            skip_group_check=True,
                        )

                def oproj_out(t, n2, acc, b=b):
                    cs = slice(512 * n2, 512 * (n2 + 1))
                    ob = opool.tile([P, 512], F32, tag="ob",
                                    name=f"ob{b}_{t}_{n2}")
                    nc.vector.tensor_add(ob[:], acc, borep_sb[:, cs])
                    nc.sync.dma_start(
                        out[b, 128 * t:128 * (t + 1), cs], ob[:])

                if b + 1 < BL:
                    # defer ALL blocks into the next batch's attention
                    # stream; the next batch's pair-0 projections (emitted
                    # right after this) give the PE dense work while this
                    # batch's last normalize chain runs
                    def full_block(t, n2, b=b, yTp=yTp):
                        o2 = psc.tile([P, 512], F32, tag="pc",
                                      name=f"o2_{b}_{t}_{n2}")
                        oproj_mm(t, n2, o2[:], 0, ET, b=b, yTp=yTp)
                        oproj_out(t, n2, o2[:], b=b)
                    deferred_op = [
                        (lambda t=t, n2=n2: full_block(t, n2))
                        for t in range(ET) for n2 in range(2)]
                else:
                    # final batch: pre-start 8 blocks' j=0..5 accumulation
                    # (4 psc slots + 4 halves of the freed PV PSUM banks) as
                    # dense PE cover while the pair-6/7 normalize chains
                    # complete; j=6,7 + bias + DMA finish afterwards
                    order = [(t, n2) for t in range(ET) for n2 in range(2)]
                    accs = [psc.tile([P, 512], F32, tag="pc",
                                     name=f"oacc{k}")[:] for k in range(4)]
                    for k in range(2):
                        bigt = pso.tile([P, 1024], F32, tag="op",
                                        name=f"obig{k}")
                        accs.append(bigt[:, 0:512])
                        accs.append(bigt[:, 512:1024])
                    for k, (t, n2) in enumerate(order[:8]):
                        oproj_mm(t, n2, accs[k], 0, 6)
                        drain(2)
                    drain(len(pending))
                    for k, (t, n2) in enumerate(order[:8]):
                        oproj_mm(t, n2, accs[k], 6, ET)
                        oproj_out(t, n2, accs[k])
                    for (t, n2) in order[8:]:
                        o2 = psc.tile([P, 512], F32, tag="pc",
                                      name=f"o2f_{t}_{n2}")
                        oproj_mm(t, n2, o2[:], 0, ET)
                        oproj_out(t, n2, o2[:])
            drain(len(pending))

    nc.compile()
    return nc


def _get_nc():
    if "nc" not in _CACHE:
        _CACHE["nc"] = _build()
    return _CACHE["nc"]


def _prep_in_maps(x, Wq, Wk, Wv, Wo, bo):
    bf16 = ml_dtypes.bfloat16
    f8 = ml_dtypes.float8_e4m3
    # [B,T,E] -> [B,E,T] transposed activations; bf16 copy feeds the V
    # projection (the value path is fp8-intolerant), fp8 copy feeds Q/K
    xT = np.ascontiguousarray(np.asarray(x).transpose(0, 2, 1)).astype(bf16)
    xT8 = xT.astype(f8)
    # [H,E,Dh] -> [E, H*Dh] (heads side by side so a 128-col slice = 2 heads)
    # Wq/Wk scaled by 32 into fp8 (see WSCALE note at top); the descale and
    # 1/sqrt(Dh) ride the exp activation's scale operand
    wq_pk = np.ascontiguousarray(
        np.asarray(Wq).transpose(1, 0, 2).reshape(E, H * Dh) * WSCALE
    ).astype(f8)
    wk_pk = np.ascontiguousarray(
        np.asarray(Wk).transpose(1, 0, 2).reshape(E, H * Dh) * WSCALE
    ).astype(f8)
    wv_pk = np.ascontiguousarray(
        np.asarray(Wv).transpose(1, 0, 2).reshape(E, H * Dh)).astype(bf16)
    wo_b = np.ascontiguousarray(np.asarray(Wo)).astype(bf16)
    borep = np.ascontiguousarray(
        np.broadcast_to(np.asarray(bo, np.float32), (P, E))).astype(bf16)
    ii, jj = np.mgrid[0:P, 0:P]
    mask01 = (jj >= ii).astype(bf16)  # S^T[tk,tq]: keep tq >= tk

    in_maps = []
    for c in range(NCORES):
        in_maps.append({
            "xT": xT[BL * c:BL * (c + 1)],
            "xT8": xT8[BL * c:BL * (c + 1)],
            "wq": wq_pk, "wk": wk_pk, "wv": wv_pk, "wo": wo_b,
            "borep": borep, "mask01": mask01,
        })
    return in_maps


def run(inputs, trace=False):
    """Returns (full_output [B,T,E] fp32, BassKernelResults)."""
    nc = _get_nc()
    in_maps = _prep_in_maps(**inputs)
    res = run_bass_kernel_spmd(nc, in_maps, core_ids=list(range(NCORES)),
                               trace=trace)
    out = np.concatenate([res.results[c]["out"] for c in range(NCORES)],
                         axis=0)
    return out, res


def kernel(x, Wq, Wk, Wv, Wo, bo):
    out, _ = run(dict(x=x, Wq=Wq, Wk=Wk, Wv=Wv, Wo=Wo, bo=bo))
    return out



# revision 56
# speedup vs baseline: 1.2452x; 1.2452x over previous
"""Causal multi-head attention (B=16, T=1024, E=1024, H=16, Dh=64) on 8 TRN2
NeuronCores.

Sharding: data-parallel over batch -- 2 batch elements per core, weights
replicated, no collectives. Host pre-transposes x and packs weights; each core
runs an identical Bass/Tile program on its shard.

Per-core dataflow (all in "transposed" orientation so no on-chip transposes
are ever needed):
  x^T [E,T] (host)   --matmul-->  Q^T,K^T [Dh,T] per head (head-pairs packed
                                  into 128 partitions; 1/sqrt(Dh) folded into
                                  the Q PSUM->SBUF copy)
                     --matmul-->  V [T,Dh] per head (+ ones column)
  S^T[tk,tq] = (K^T tile).T @ Q^T  per key-tile, causal tiles skipped
  P^T = exp(S^T) on ScalarE (scores are O(1): no max subtraction needed);
        diagonal tiles masked by 0/1 multiply
  O'^T[65,tq] += (V'|1).T @ P^T   -- row 64 accumulates the softmax denom
  Y^T = O'^T[0:64] * bcast(1/denom)
  out[t,E] = Y^T.T @ Wo + bo

Scheduling notes (hard-won on HW):
  - engine APs need partition base in {0,32,64,96}; partition_broadcast reads
    physical partition 0 regardless of the AP; gpsimd + custom-DVE ops
    (reciprocal_approx_fast) silently misread NONZERO partition bases on HW
    (CoreSim hides it), and approx_fast must also be out-of-place.
  - HAM unthrottles the PE (1.2 -> 2.4 GHz) only on a fully-busy 3.4us
    window, so each pair's Q/K projection matmuls, the previous batch's
    out-proj blocks (3/pair over pairs 0-5) and the next batch's x-load +
    V-projection (pairs 6-7) are drip-fed into the attention stream as
    dense PE filler.
  - the attention sub-block loop is software-pipelined a half step (scores
    for sub n+1 issue before PV of sub n) to cover the exp latency; deeper
    lookahead deadlocks the Tile scheduler's finite wait queues.
  - all PSUM->SBUF copies run on the DVE (tensor_copy), keeping the scalar
    engine dedicated to the exp stream, which paces the attention phase.
  - normalization (approx-reciprocal + stage + broadcast + 256-col muls) is
    drip-emitted between sub-steps; the final batch pre-starts 8 out-proj
    j=0..5 accumulations (4 psc + 4 halves of freed PV PSUM banks) as PE
    cover for the last normalize chains.
"""
import numpy as np
import ml_dtypes

import concourse.bass as bass
import concourse.mybir as mybir
import concourse.tile as tile
from concourse import bacc
from concourse.bass_utils import run_bass_kernel_spmd

B, T, E = 16, 1024, 1024
H, Dh = 16, 64
NCORES = 8
BL = B // NCORES          # batches per core
P = 128                   # partitions
ET = E // P               # 8 tiles along E / token / hd dims
HP = H // 2               # 8 head-pairs
BF = mybir.dt.bfloat16
F8 = mybir.dt.float8e4
F32 = mybir.dt.float32
AF = mybir.ActivationFunctionType
DR = mybir.MatmulPerfMode.DoubleRow
# Wq/Wk are uploaded as fp8 scaled by 32 (half their values sit below
# e4m3's normal range otherwise); 0.125 = 1/sqrt(Dh) is NOT folded into
# Wq anymore.  S' = (32 q)·(32 k) = 1024·qk, so exp gets scale
# 0.125/1024 for free via the activation scale operand.
WSCALE = 32.0
ESCALE = 0.125 / (WSCALE * WSCALE)

_CACHE = {}


def _pieces(i):
    """Column pieces of [128*i, 1024) that do not cross the 512 PSUM-bank
    boundary."""
    if i < 4:
        return [(128 * i, 512), (512, 1024)]
    return [(128 * i, 1024)]


def _build(dbg=False):
    nc = bacc.Bacc("TRN2", target_bir_lowering=False, debug=False,
                   num_devices=NCORES)

    dbg_out = {}
    if dbg:
        for name, shape, dt in [
            ("d_qT", [P, HP, T], BF), ("d_kT", [P, HP, T], BF),
            ("d_v", [P, ET, H, Dh + 1], BF), ("d_pt", [ET, P, T], BF),
            ("d_op", [P, T], F32),
            ("d_r1", [1, T], F32), ("d_rb", [Dh, T], F32),
            ("d_yT", [P, HP, T], BF),
        ]:
            dbg_out[name] = nc.dram_tensor(name, shape, dt,
                                           kind="ExternalOutput").ap()

    xT = nc.dram_tensor("xT", [BL, E, T], BF, kind="ExternalInput").ap()
    xT8 = nc.dram_tensor("xT8", [BL, E, T], F8, kind="ExternalInput").ap()
    wq = nc.dram_tensor("wq", [E, H * Dh], F8, kind="ExternalInput").ap()
    wk = nc.dram_tensor("wk", [E, H * Dh], F8, kind="ExternalInput").ap()
    wv = nc.dram_tensor("wv", [E, H * Dh], BF, kind="ExternalInput").ap()
    wo = nc.dram_tensor("wo", [H * Dh, E], BF, kind="ExternalInput").ap()
    borep = nc.dram_tensor("borep", [P, E], BF, kind="ExternalInput").ap()
    mask01 = nc.dram_tensor("mask01", [P, P], BF, kind="ExternalInput").ap()
    out = nc.dram_tensor("out", [BL, T, E], F32, kind="ExternalOutput").ap()

    with tile.TileContext(nc) as tc:
        with (
            tc.tile_pool(name="consts", bufs=1) as cpool,
            tc.tile_pool(name="xp", bufs=2) as xpool,
            tc.tile_pool(name="xp8", bufs=2) as x8pool,
            tc.tile_pool(name="qk", bufs=2) as qkpool,
            tc.tile_pool(name="vp2", bufs=2) as vpool,
            tc.tile_pool(name="vy", bufs=2) as vypool,
            tc.tile_pool(name="pt", bufs=6) as ptpool,
            tc.tile_pool(name="sm", bufs=2) as spool,
            tc.tile_pool(name="dn", bufs=1) as dnpool,
            tc.tile_pool(name="ob", bufs=3) as opool,
            tc.tile_pool(name="pso", bufs=2, space="PSUM") as pso,
            tc.tile_pool(name="psc", bufs=4, space="PSUM") as psc,
        ):
            # DMA order matters for the startup critical path: V-projection
            # only needs Wv + xT, so those go first; Wo/bias are not needed
            # until the output projection
            wq_sb = cpool.tile([P, ET, H * Dh], F8, tag="wq")
            wk_sb = cpool.tile([P, ET, H * Dh], F8, tag="wk")
            wv_sb = cpool.tile([P, ET, H * Dh], BF, tag="wv")
            wo_sb = cpool.tile([P, ET, E], BF, tag="wo")
            # startup order: V-path inputs (wv in halves, then xT in
            # t-chunks) go first -- V-projection blocks are the PE's
            # startup filler and the first one only needs wv's first half
            # plus xT chunk 0; Q/K-proj inputs stream in underneath
            wvr = wv.rearrange("(n p) c -> p n c", p=P)
            nc.sync.dma_start(wv_sb[:, :, 0:512], wvr[:, :, 0:512])

            xT_tiles = {}
            xT8_tiles = {}
            v_tiles = {}

            def load_blocks(b):
                """xT load + V-projection for batch b as dense PE filler
                blocks (also usable as pending entries during the previous
                batch's last attention pairs)."""
                def ld(b=b):
                    # xT arrives in t-chunks so V block t only waits for
                    # its own chunk (first V matmul ~8us instead of ~24us)
                    xT_tiles[b] = xpool.tile([P, ET, T], BF, tag="xT",
                                             name=f"xT{b}")
                    xsrc = xT[b].rearrange("(n p) c -> p n c", p=P)
                    for t in range(ET):
                        ts_ = slice(128 * t, 128 * (t + 1))
                        nc.sync.dma_start(xT_tiles[b][:, :, ts_],
                                          xsrc[:, :, ts_])
                    xT8_tiles[b] = x8pool.tile([P, ET, T], F8, tag="xT8",
                                               name=f"xT8_{b}")
                    nc.sync.dma_start(
                        xT8_tiles[b][:],
                        xT8[b].rearrange("(n p) c -> p n c", p=P))
                    v_tiles[b] = vpool.tile([P, ET, H, Dh + 1], BF, tag="v",
                                            name=f"v{b}")
                    nc.vector.memset(v_tiles[b][:, :, :, Dh], 1.0)
                blocks = [ld]
                for n2 in range(2):
                    for t in range(ET):
                        def vblk(t=t, n2=n2, b=b):
                            cs = slice(512 * n2, 512 * (n2 + 1))
                            vp = psc.tile([P, 512], F32, tag="pc",
                                          name=f"vp{b}_{t}_{n2}")
                            for i in range(ET):
                                nc.tensor.matmul(
                                    vp[:],
                                    lhsT=xT_tiles[b][:, i,
                                                     128 * t:128 * (t + 1)],
                                    rhs=wv_sb[:, i, cs],
                                    start=(i == 0), stop=(i == ET - 1),
                                )
                            nc.vector.tensor_copy(
                                v_tiles[b][:, t, 8 * n2:8 * (n2 + 1), 0:Dh],
                                vp[:].rearrange("p (h d) -> p h d", d=Dh),
                            )
                        blocks.append(vblk)
                return blocks

            blocks0 = load_blocks(0)
            blocks0[0]()  # xT chunks + x8 right behind Wv's first half
            nc.sync.dma_start(wv_sb[:, :, 512:1024], wvr[:, :, 512:1024])
            nc.sync.dma_start(wq_sb[:], wq.rearrange("(n p) c -> p n c", p=P))
            nc.sync.dma_start(wk_sb[:], wk.rearrange("(n p) c -> p n c", p=P))
            mask_sb = cpool.tile([P, P], BF, tag="mask")
            nc.sync.dma_start(mask_sb[:], mask01)
            for blk in blocks0[1:]:
                blk()
            nc.sync.dma_start(wo_sb[:], wo.rearrange("(n p) c -> p n c", p=P))
            borep_sb = cpool.tile([P, E], BF, tag="bo")
            nc.sync.dma_start(borep_sb[:], borep)

            pending = []

            def drain(n):
                for _ in range(min(n, len(pending))):
                    pending.pop(0)()

            qk_tiles = {}
            deferred_op = []

            for b in range(BL):
                xT_sb = xT_tiles[b]
                v_sb = v_tiles[b]

                # ---- Q^T / K^T projections, emitted as closures so pair
                # pp's projection interleaves into pair pp-1's attention.
                # Per-pair [P,T] tiles (bufs=2): finer Tile deps + 24KB SBUF
                # saved vs whole-batch [P,HP,T] tiles ----
                def proj_subblocks(pp, b=b):
                    # fp8 DoubleRow: each pass consumes TWO 128-deep E tiles
                    # (lhsT [128,2,128], rhs [128,2,512]), halving PE time
                    # vs bf16.  Q^T/K^T land in bf16 [P,T] per head-pair;
                    # the scores matmul stays bf16 (a single 64-deep pass --
                    # DoubleRow only pays when it halves accumulation
                    # passes, so fp8 there buys nothing).
                    blocks = []
                    for (lbl, w_sb) in (("q", wq_sb), ("k", wk_sb)):
                        for n2 in range(2):
                            def blk(lbl=lbl, w_sb=w_sb, n2=n2, pp=pp, b=b):
                                if (b, pp) not in qk_tiles:
                                    qk_tiles[(b, pp)] = (
                                        qkpool.tile([P, T], BF, tag="q",
                                                    name=f"q{b}_{pp}"),
                                        qkpool.tile([P, T], BF, tag="k",
                                                    name=f"k{b}_{pp}"))
                                dst = qk_tiles[(b, pp)][0 if lbl == "q"
                                                        else 1]
                                cs = slice(512 * n2, 512 * (n2 + 1))
                                x8_sb = xT8_tiles[b]
                                pj = psc.tile(
                                    [P, 512], F32, tag="pc",
                                    name=f"pj{b}_{pp}_{n2}_{lbl}")
                                for i in range(0, ET, 2):
                                    nc.tensor.matmul(
                                        pj[:],
                                        lhsT=w_sb[:, i:i + 2,
                                                  128 * pp:128 * (pp + 1)],
                                        rhs=x8_sb[:, i:i + 2, cs],
                                        start=(i == 0), stop=(i == ET - 2),
                                        perf_mode=DR,
                                    )
                                nc.vector.tensor_copy(dst[:, cs], pj[:])
                            blocks.append(blk)
                    return blocks

                for blk in proj_subblocks(0):
                    blk()

                # ---- attention: pairs of heads, drip-scheduled extras ----
                # per-pair yT tiles, double-buffered across batches: out-proj
                # j only waits on pair j's normalize, and the previous
                # batch's deferred out-proj never blocks this batch's writes
                yTp = [vypool.tile([P, T], BF, tag=f"y{hp}",
                                   name=f"y{b}_{hp}") for hp in range(HP)]

                def yT_ap(hp):
                    return yTp[hp][:, :]
                # den: PV-accumulated softmax denominators (2 rotating quad
                # slots); den_r: their reciprocals. approx_fast is ~5x faster
                # than reciprocal() but MUST be out-of-place: in-place it
                # returns garbage on HW (verified; CoreSim hides it)
                den = dnpool.tile([P, 2, T], F32, tag="den")
                den_r = dnpool.tile([P, 2, T], F32, tag="denr")
                nc.vector.memset(den[:], 1.0)

                def queue_normalize(g, b=b, yT_ap=yT_ap, den=den,
                                    half=None):
                    # half=0/1 reciprocates only partitions [0:64)/[64:128)
                    # of the slot (heads 4g..4g+1 / 4g+2..4g+3) so the last
                    # quad can normalize pair-by-pair instead of in one
                    # end-of-batch burst
                    # approx_fast also breaks on HW with a nonzero partition
                    # offset, so always recip the full 128 partitions; for
                    # the half= calls the other half recomputes identical
                    # values (benign)
                    for c in range(4):
                        def recip_chunk(g=g, c=c):
                            nc.vector.reciprocal_approx_fast(
                                den_r[0:P, g % 2, 256 * c:256 * (c + 1)],
                                den[0:P, g % 2, 256 * c:256 * (c + 1)])
                        pending.append(recip_chunk)
                    if dbg and b == 0 and g == 0 and half is None:
                        pending.append(lambda: nc.sync.dma_start(
                            dbg_out["d_r1"], den[0:1, 0, :]))
                    heads = range(4 * g, 4 * g + 4) if half is None else \
                        range(4 * g + 2 * half, 4 * g + 2 * half + 2)
                    for h in heads:
                        holder = {}

                        def stage(h=h, b=b, holder=None):
                            hp, po = h // 2, Dh * (h % 2)
                            pb = 32 * (h % 4)
                            r1 = spool.tile([1, T], BF, tag="r1",
                                            name=f"r1_{b}_{h}")
                            nc.scalar.activation(
                                r1[:], den_r[pb:pb + 1, (h // 4) % 2, :],
                                AF.Copy)
                            rb = spool.tile([P, T], BF, tag="rb",
                                            name=f"rb_{b}_{h}")
                            nc.gpsimd.partition_broadcast(rb[:], r1[:])
                            holder['rb'] = rb

                        pending.append(
                            lambda h=h, holder=holder: stage(h, b, holder))
                        # normalize muls chunked to 256 cols so no single
                        # DVE op delays the mask->PV chain by more than
                        # ~0.5us
                        for c in range(4):
                            def mul_chunk(h=h, holder=holder, c=c):
                                hp, po = h // 2, Dh * (h % 2)
                                ap = yT_ap(hp)
                                sl = slice(256 * c, 256 * (c + 1))
                                nc.vector.tensor_mul(
                                    ap[po:po + Dh, sl],
                                    ap[po:po + Dh, sl],
                                    holder['rb'][po:po + Dh, sl])
                            pending.append(mul_chunk)

                for hp in range(HP):
                    if deferred_op and hp >= 1:
                        # previous batch's out-proj blocks as dense PE
                        # filler -- NOT at pair 0: every block reads all 8
                        # yT pairs, and the previous batch's last normalize
                        # only clears during pair 0, so eager drips stall
                        # the PE there
                        take = 3 if hp <= 2 else 2
                        pending.extend(deferred_op[:take])
                        del deferred_op[:take]
                    if hp + 1 < HP:
                        # front of the queue: pair hp+1's projection must
                        # finish within this pair's attention
                        pending[0:0] = proj_subblocks(hp + 1)
                    if b + 1 < BL:
                        # next batch's x load + V projection: V blocks have
                        # no upstream compute deps, so they carry pair 0
                        # (where out-proj can't run yet) and then 2/pair
                        if hp == 0:
                            next_blocks = load_blocks(b + 1)
                            pending.extend(next_blocks[:5])
                        else:
                            pending.extend(next_blocks[2 * hp + 3:
                                                       2 * hp + 5])
                    ops = [pso.tile([P, 1024], F32, tag="op",
                                    name=f"op{b}_{hp}_{s}") for s in range(2)]

                    def emit_scores_sub(i, sub, hp=hp, b=b):
                        # per-piece P tiles: the PV matmul for the non-
                        # diagonal piece only waits on its own exp, not on
                        # the diagonal piece's exp+mask
                        qTp, kTp = qk_tiles[(b, hp)]
                        po = Dh * sub
                        pts = []
                        for (a0, a1) in _pieces(i):
                            w = a1 - a0
                            ptp = ptpool.tile([P, 512], BF, tag="pt",
                                              name=f"pt{b}_{hp}_{i}_{sub}_{a0}")
                            sp_ = psc.tile([P, 512], F32, tag="pc",
                                           name=f"sp{b}_{hp}_{i}_{sub}_{a0}")
                            nc.tensor.matmul(
                                sp_[:, 0:w],
                                lhsT=kTp[po:po + Dh,
                                         128 * i:128 * (i + 1)],
                                rhs=qTp[po:po + Dh, a0:a1],
                                start=True, stop=True,
                            )
                            nc.scalar.activation(ptp[:, 0:w],
                                                 sp_[:, 0:w], AF.Exp,
                                                 scale=ESCALE)
                            if a0 == 128 * i:
                                # diagonal 128-col block lives at the start
                                # of the first piece
                                nc.vector.tensor_mul(ptp[:, 0:128],
                                                     ptp[:, 0:128],
                                                     mask_sb[:])
                            pts.append((a0, a1, ptp))
                        return pts

                    def emit_pv_sub(i, sub, pts, hp=hp, ops=ops, v_sb=v_sb,
                                    b=b):
                        h = 2 * hp + sub
                        for (a0, a1, ptp) in pts:
                            nc.tensor.matmul(
                                ops[sub][0:Dh + 1, a0:a1],
                                lhsT=v_sb[:, i, h, :],
                                rhs=ptp[:, 0:a1 - a0],
                                start=(i == 0), stop=(i == ET - 1),
                                skip_group_check=True,
                            )
                        if i == ET - 1:
                            # yT+den copies dripped into the next pair's
                            # drain stream: emitted inline they burst ~4us
                            # of DVE right when the next pair's mask muls
                            # need it (the pso pool is double-buffered, so
                            # the psum only has to free a pair later)
                            def pv_out_copies(hp=hp, sub=sub, h=h,
                                              ops=ops):
                                po = Dh * sub
                                nc.vector.tensor_copy(
                                    yT_ap(hp)[po:po + Dh, :],
                                    ops[sub][0:Dh, :])
                                pb = 32 * (h % 4)
                                nc.vector.tensor_copy(
                                    den[pb:pb + 1, (h // 4) % 2, :],
                                    ops[sub][Dh:Dh + 1, :])
                            pending.append(pv_out_copies)
                            if dbg and b == 0 and h == 0:
                                opc = dnpool.tile([P, T], F32,
                                                  tag="dbg_op")
                                nc.vector.tensor_copy(opc[:],
                                                      ops[sub][:])
                                nc.sync.dma_start(dbg_out["d_op"],
                                                  opc[:])

                    # half-step software pipeline: scores for sub-block n+1
                    # issue on the PE before the PV matmuls of sub-block n,
                    # covering the exp+mask latency without over-filling the
                    # engines' wait queues (full-step lookahead deadlocks the
                    # Tile scheduler)
                    dr = 2 if (b == BL - 1 and hp >= HP - 2) else 1
                    prev = None
                    for i in range(ET):
                        for sub in (0, 1):
                            pt = emit_scores_sub(i, sub)
                            drain(dr)
                            if prev is not None:
                                emit_pv_sub(*prev)
                            prev = (i, sub, pt)
                    drain(dr)
                    emit_pv_sub(*prev)
                    if hp == HP - 2:
                        queue_normalize(3, half=0)
                    elif hp == HP - 1:
                        queue_normalize(3, half=1)
                    elif hp % 2 == 1:
                        queue_normalize(hp // 2)

                # emission order IS dependency order under Tile's tracer:
                # all normalize muls must be emitted before out-proj reads yT
                drain(len(pending))

                # ---- output projection + bias ----
                def oproj_mm(t, n2, acc, j0, j1, b=b, yTp=yTp):
                    cs = slice(512 * n2, 512 * (n2 + 1))
                    for j in range(j0, j1):
                        nc.tensor.matmul(
                            acc,
                            lhsT=yTp[j][:, 128 * t:128 * (t + 1)],
                            rhs=wo_sb[:, j, cs],
                            start=(j == 0), stop=(j == ET - 1),
                            skip_group_check=True,
                        )

                def oproj_out(t, n2, acc, b=b):
                    cs = slice(512 * n2, 512 * (n2 + 1))
                    ob = opool.tile([P, 512], F32, tag="ob",
                                    name=f"ob{b}_{t}_{n2}")
                    nc.vector.tensor_add(ob[:], acc, borep_sb[:, cs])
                    nc.sync.dma_start(
                        out[b, 128 * t:128 * (t + 1), cs], ob[:])

                if b + 1 < BL:
                    # defer ALL blocks into the next batch's attention
                    # stream; the next batch's pair-0 projections (emitted
                    # right after this) give the PE dense work while this
                    # batch's last normalize chain runs
                    def full_block(t, n2, b=b, yTp=yTp):
                        o2 = psc.tile([P, 512], F32, tag="pc",
                                      name=f"o2_{b}_{t}_{n2}")
                        oproj_mm(t, n2, o2[:], 0, ET, b=b, yTp=yTp)
                        oproj_out(t, n2, o2[:], b=b)
                    deferred_op = [
                        (lambda t=t, n2=n2: full_block(t, n2))
                        for t in range(ET) for n2 in range(2)]
                else:
                    # final batch: pre-start 8 blocks' j=0..5 accumulation
                    # (4 psc slots + 4 halves of the freed PV PSUM banks) as
                    # dense PE cover while the pair-6/7 normalize chains
                    # complete; j=6,7 + bias + DMA finish afterwards
                    order = [(t, n2) for t in range(ET) for n2 in range(2)]
                    accs = [psc.tile([P, 512], F32, tag="pc",
                                     name=f"oacc{k}")[:] for k in range(4)]
                    for k in range(2):
                        bigt = pso.tile([P, 1024], F32, tag="op",
                                        name=f"obig{k}")
                        accs.append(bigt[:, 0:512])
                        accs.append(bigt[:, 512:1024])
                    for k, (t, n2) in enumerate(order[:8]):
                        oproj_mm(t, n2, accs[k], 0, 6)
                        drain(2)
                    drain(len(pending))
                    for k, (t, n2) in enumerate(order[:8]):
                        oproj_mm(t, n2, accs[k], 6, ET)
                        oproj_out(t, n2, accs[k])
                    for (t, n2) in order[8:]:
                        o2 = psc.tile([P, 512], F32, tag="pc",
                                      name=f"o2f_{t}_{n2}")
                        oproj_mm(t, n2, o2[:], 0, ET)
                        oproj_out(t, n2, o2[:])
            drain(len(pending))

    nc.compile()
    return nc


def _get_nc():
    if "nc" not in _CACHE:
        _CACHE["nc"] = _build()
    return _CACHE["nc"]


def _prep_in_maps(x, Wq, Wk, Wv, Wo, bo):
    bf16 = ml_dtypes.bfloat16
    f8 = ml_dtypes.float8_e4m3
    # [B,T,E] -> [B,E,T] transposed activations; bf16 copy feeds the V
    # projection (the value path is fp8-intolerant), fp8 copy feeds Q/K
    xT = np.ascontiguousarray(np.asarray(x).transpose(0, 2, 1)).astype(bf16)
    xT8 = xT.astype(f8)
    # [H,E,Dh] -> [E, H*Dh] (heads side by side so a 128-col slice = 2 heads)
    # Wq/Wk scaled by 32 into fp8 (see WSCALE note at top); the descale and
    # 1/sqrt(Dh) ride the exp activation's scale operand
    wq_pk = np.ascontiguousarray(
        np.asarray(Wq).transpose(1, 0, 2).reshape(E, H * Dh) * WSCALE
    ).astype(f8)
    wk_pk = np.ascontiguousarray(
        np.asarray(Wk).transpose(1, 0, 2).reshape(E, H * Dh) * WSCALE
    ).astype(f8)
    wv_pk = np.ascontiguousarray(
        np.asarray(Wv).transpose(1, 0, 2).reshape(E, H * Dh)).astype(bf16)
    wo_b = np.ascontiguousarray(np.asarray(Wo)).astype(bf16)
    borep = np.ascontiguousarray(
        np.broadcast_to(np.asarray(bo, np.float32), (P, E))).astype(bf16)
    ii, jj = np.mgrid[0:P, 0:P]
    mask01 = (jj >= ii).astype(bf16)  # S^T[tk,tq]: keep tq >= tk

    in_maps = []
    for c in range(NCORES):
        in_maps.append({
            "xT": xT[BL * c:BL * (c + 1)],
            "xT8": xT8[BL * c:BL * (c + 1)],
            "wq": wq_pk, "wk": wk_pk, "wv": wv_pk, "wo": wo_b,
            "borep": borep, "mask01": mask01,
        })
    return in_maps


def run(inputs, trace=False):
    """Returns (full_output [B,T,E] fp32, BassKernelResults)."""
    nc = _get_nc()
    in_maps = _prep_in_maps(**inputs)
    res = run_bass_kernel_spmd(nc, in_maps, core_ids=list(range(NCORES)),
                               trace=trace)
    out = np.concatenate([res.results[c]["out"] for c in range(NCORES)],
                         axis=0)
    return out, res


def kernel(x, Wq, Wk, Wv, Wo, bo):
    out, _ = run(dict(x=x, Wq=Wq, Wk=Wk, Wv=Wv, Wo=Wo, bo=bo))
    return out

